# revision 2
# baseline (speedup 1.0000x reference)
"""GAT layer (nn_GATLayer) on 8 Trainium2 NeuronCores — AND-mask formulation.

Math (reference):
    Wh = X @ weight;  s = Wh @ a[:F];  t = Wh @ a[F:]
    att = softmax(where(A > 0, relu(s_i + t_j), -9e15), axis=1)
    out = elu(att @ Wh)

Kernel formulation (shift-free softmax, exact up to fp rounding):
    es_i = c*exp(s_i) (bf16 row), et_j = exp(t_j) (fp32 col)
    p_ij = A_ij * max(es_i*et_j, c)    [= c*e^{relu(s+t)} on edges]
    out_i = elu((p_i: @ Wh) / sum_j p_ij)

Per-pair mask application, three ISA-valid engine paths:
  R-pairs (DVE): z8 = fp8 max(es*et, c) via tensor_scalar 2x_2p, then
    p8 = z8 & mask as a uint32 bitwise AND (4 fp8 lanes per u32 lane,
    quarter free-size) — mask bytes are {0x00, 0xFF}.
  A-pairs (ACT+DVE): z8 = fp8 c*exp(relu(s_bc + t_j)) via two ACT ops
    (Relu with t-bias ptr, then Exp with ln c bias), then the same AND.
  C-pairs (DVE+Pool): rt = bf16 max(es*et, c) via tensor_scalar 4x, then
    p8 = rt * mask on Pool (tensor_tensor mult) — mask is {0, 1} so all
    pair classes produce identically-scaled p8.
The host encodes the mask slab per pair class (byte-mask rows for R/A
pairs, {0,c} fp8 rows for C-pairs) — dtype/layout prep only. Mask DMAs
are per-pair for fine-grained pipeline flow.

num = Wh8^T @ p8 and den = 1^T @ p8 accumulate in PSUM over all 64
j-tiles (fp8 DoubleRow, 4 matmuls per pair); epilogue is reciprocal +
PE transposes + fused ELU, output DMA'd per 256-row slab.

Sharding: 1D row partition across 8 cores (1024 rows each); X^T bf16
grouped + rotated so group 0 is own rows; weight bf16 replicated.
"""

import numpy as np
import ml_dtypes

import concourse.bass as bass
import concourse.bacc as bacc
import concourse.mybir as mybir
import concourse.tile as tile
from concourse.bass_utils import run_bass_kernel_spmd

N = 8192
F_IN = 512
F_OUT = 128
N_CORES = 8
ROWS = N // N_CORES          # 1024 rows per core
NT = N // 128                # 64 j tiles
NP = NT // 2                 # 32 j tile pairs
KC = F_IN // 128             # 4 f_in chunks
NG = 8                       # XT groups (8 j-tiles each)

C = 0.25                     # clamp value / global scale
LNC = float(np.log(C))

FP32 = mybir.dt.float32
BF16 = mybir.dt.bfloat16
FP8 = mybir.dt.float8e4
U32 = mybir.dt.uint32
Alu = mybir.AluOpType
Act = mybir.ActivationFunctionType
DR = mybir.MatmulPerfMode.DoubleRow

# engine assignment per pair (tuned against TimelineSim). Pool pairs are
# front-loaded: their masks arrive early and Pool's slow multiplies must
# not become the tail straggler.
POOL_PAIRS = frozenset({2, 5, 7, 11, 14, 17, 20, 23, 26, 28})
ACT_PAIRS = frozenset({3, 8, 13, 18, 24, 30})

_cache = {}


def _build():
    nc = bacc.Bacc("TRN2", target_bir_lowering=False, debug=False,
                   num_devices=N_CORES)

    XTg = nc.dram_tensor("XTg", [NG, KC, 128, ROWS], BF16, kind="ExternalInput")
    MK8 = nc.dram_tensor("MK8", [N, ROWS], FP8, kind="ExternalInput")
    w_in = nc.dram_tensor("w_in", [F_IN, F_OUT], BF16, kind="ExternalInput")
    a_vec = nc.dram_tensor("a_vec", [2 * F_OUT, 1], FP32, kind="ExternalInput")
    ident = nc.dram_tensor("ident", [128, 128], FP32, kind="ExternalInput")
    out_d = nc.dram_tensor("out", [ROWS, F_OUT], FP32, kind="ExternalOutput")

    with tile.TileContext(nc) as tc:
        _body(nc, tc, XTg, MK8, w_in, a_vec, ident, out_d)

    nc.compile()
    return nc


def _body(nc, tc, XTg, MK8, w_in, a_vec, ident, out_d):
    with (
        tc.tile_pool(name="setup", bufs=1) as setup,
        tc.tile_pool(name="xtg", bufs=4) as xtg_pool,
        tc.tile_pool(name="mk", bufs=1) as mk_pool,
        tc.tile_pool(name="mm", bufs=10) as mm_pool,
        tc.tile_pool(name="epi", bufs=1) as epi,
    ):
        # ---------------- setup: weights ----------------
        psW_cm = tc.tile_pool(name="psW", bufs=1, space="PSUM")
        psW = psW_cm.__enter__()

        w_sb = setup.tile([128, KC, 128], BF16)
        nc.sync.dma_start(
            out=w_sb, in_=w_in.rearrange("(k p) f -> p k f", p=128)
        )
        idn = setup.tile([128, 128], FP32)
        nc.sync.dma_start(out=idn, in_=ident[:, :])
        a_cat = setup.tile([128, 2], BF16)
        nc.gpsimd.dma_start(
            out=a_cat, in_=a_vec.rearrange("(h p) o -> p (h o)", p=128)
        )
        idn_bf = setup.tile([128, 128], BF16)
        nc.vector.tensor_copy(idn_bf, idn)

        # first XT group doubles as own-rows X^T (host rotates groups)
        xtg0 = xtg_pool.tile([128, KC, ROWS], BF16, tag="g0")
        for k in range(KC):
            nc.sync.dma_start(
                out=xtg0[:, k, :],
                in_=XTg[0].rearrange("k p i -> p k i")[:, k, :],
            )

        # w_all[:, k, :] = [weight_k | w_t_k | w_s_k]  [128, KC, 130]
        w_all = setup.tile([128, KC, 130], BF16)
        for k in range(KC):
            nc.vector.tensor_copy(w_all[:, k, 0:F_OUT], w_sb[:, k, :])
            ps_wT = psW.tile([128, 128], BF16, tag="wT")
            nc.tensor.transpose(ps_wT, w_sb[:, k, :], idn_bf)
            wT = setup.tile([128, 128], BF16, tag=f"wT{k}")
            nc.vector.tensor_copy(wT, ps_wT)
            ps_a = psW.tile([128, 2], FP32, tag="pa")
            nc.tensor.matmul(ps_a, wT, a_cat, start=True, stop=True)
            # col F_OUT = w_t (a[F:]), col F_OUT+1 = w_s (a[:F])
            nc.vector.tensor_copy(w_all[:, k, F_OUT : F_OUT + 1], ps_a[:, 1:2])
            nc.vector.tensor_copy(
                w_all[:, k, F_OUT + 1 : F_OUT + 2], ps_a[:, 0:1]
            )
        psW_cm.__exit__(None, None, None)

        # ---------------- s (own rows) + es/s broadcasts ----------------
        psA_cm = tc.tile_pool(name="psAset", bufs=1, space="PSUM")
        psAs = psA_cm.__enter__()

        ps_s = psAs.tile([1, ROWS], FP32, tag="ps_s")
        for h in range(2):
            sl = slice(512 * h, 512 * (h + 1))
            for k in range(KC):
                nc.tensor.matmul(
                    ps_s[:, sl],
                    w_all[:, k, 129:130],
                    xtg0[:, k, sl],
                    start=(k == 0), stop=(k == KC - 1),
                    skip_group_check=True,
                )
        lnc1 = setup.tile([1, 1], FP32, tag="lnc1")
        nc.vector.memset(lnc1, LNC)
        es_row = setup.tile([1, ROWS], BF16)
        s_row = setup.tile([1, ROWS], BF16)
        for h in range(2):
            sl = slice(512 * h, 512 * (h + 1))
            nc.scalar.activation(
                out=es_row[:, sl], in_=ps_s[:, sl], func=Act.Exp, bias=lnc1
            )
            nc.vector.tensor_copy(s_row[:, sl], ps_s[:, sl])
        ones_r = setup.tile([1, 128], BF16)
        nc.vector.memset(ones_r, 1.0)
        es_bc = setup.tile([128, ROWS], BF16)
        s_bc = setup.tile([128, ROWS], BF16)
        for h in range(2):
            sl = slice(512 * h, 512 * (h + 1))
            ps_b = psAs.tile([128, 512], FP32, tag="bc", bufs=2)
            nc.tensor.matmul(ps_b, ones_r, es_row[:, sl], start=True, stop=True)
            nc.vector.tensor_copy(es_bc[:, sl], ps_b)
            ps_b2 = psAs.tile([128, 512], FP32, tag="bc", bufs=2)
            nc.tensor.matmul(ps_b2, ones_r, s_row[:, sl], start=True, stop=True)
            nc.scalar.copy(s_bc[:, sl], ps_b2)
        psA_cm.__exit__(None, None, None)

        # den stationary: ones in 32 columns -> den sums land in psum
        # partitions 0-31 (only row 0 is read)
        ones_m = setup.tile([128, 2, 32], FP8)
        nc.vector.memset(ones_m, 1.0)

        wh_all = setup.tile([128, NT, F_OUT], FP8)
        et_cols = setup.tile([128, NT], FP32)
        t_cols = setup.tile([128, NT], FP32)
        lncB = setup.tile([128, 1], FP32, tag="lncB")
        nc.vector.memset(lncB, LNC)

        with (
            tc.tile_pool(name="psA", bufs=1, space="PSUM") as psAp,
            tc.tile_pool(name="psDen", bufs=1, space="PSUM") as psDp,
        ):
            psA = psAp.tile([128, ROWS], FP32)
            ps_den = psDp.tile([32, ROWS], FP32)

            with tc.tile_pool(name="psS", bufs=1, space="PSUM") as psS:
                mk_tiles = {}
                xtg_tiles = {0: xtg0}

                def emit_dma(g):
                    if g > 0:
                        xtg = xtg_pool.tile([128, KC, ROWS], BF16)
                        xtg_tiles[g] = xtg
                        for k in range(KC):
                            nc.sync.dma_start(
                                out=xtg[:, k, :],
                                in_=XTg[g].rearrange("k p i -> p k i")[:, k, :],
                            )
                    for t in range(4 * g, 4 * g + 4):
                        mk = mk_pool.tile([128, 2, ROWS], FP8, tag="mk", bufs=16)
                        mk_tiles[t] = mk
                        nc.sync.dma_start(
                            out=mk,
                            in_=MK8[256 * t : 256 * (t + 1), :].rearrange(
                                "(t p) i -> p t i", p=128
                            ),
                        )

                def emit_wh(g):
                    # Wh | t for the group's 8 tiles (ps_p must stay within
                    # one 2KB PSUM bank: [128, 2, 130] = 1040B)
                    xtg = xtg_tiles[g]
                    for q in range(4):
                        jt0 = 8 * g + 2 * q
                        ps_p = psS.tile([128, 2, 130], FP32, bufs=4)
                        for v in range(2):
                            co = 128 * (2 * q + v)
                            for k in range(KC):
                                nc.tensor.matmul(
                                    ps_p[:, v, :],
                                    xtg[:, k, co : co + 128],
                                    w_all[:, k, :],
                                    start=(k == 0), stop=(k == KC - 1),
                                    skip_group_check=True,
                                )
                        nc.scalar.copy(
                            wh_all[:, jt0 : jt0 + 2, :], ps_p[:, :, 0:F_OUT]
                        )
                        nc.scalar.activation(
                            out=et_cols[:, jt0 : jt0 + 2],
                            in_=ps_p[:, :, F_OUT : F_OUT + 1],
                            func=Act.Exp,
                        )
                        nc.scalar.activation(
                            out=t_cols[:, jt0 : jt0 + 2],
                            in_=ps_p[:, :, F_OUT : F_OUT + 1],
                            func=Act.Identity,
                        )

                def emit_pair(t):
                    mksl = mk_tiles[t]
                    first, last = t == 0, t == NP - 1
                    p8 = mm_pool.tile([128, 2, ROWS], FP8, tag="p8", bufs=14)
                    if t in POOL_PAIRS:
                        # rt = bf16 max(es*et, c) at 4x; Pool multiplies
                        # by the {0, c} mask
                        rt = mm_pool.tile([128, 2, ROWS], BF16, tag="rt", bufs=3)
                        for v in range(2):
                            jt = 2 * t + v
                            nc.vector.tensor_scalar(
                                out=rt[:, v, :], in0=es_bc,
                                scalar1=et_cols[:, jt : jt + 1],
                                scalar2=C, op0=Alu.mult, op1=Alu.max,
                            )
                        nc.gpsimd.tensor_tensor(
                            out=p8, in0=rt, in1=mksl, op=Alu.mult
                        )
                    else:
                        z8 = mm_pool.tile([128, 2, ROWS], FP8, tag="z8", bufs=5)
                        if t in ACT_PAIRS:
                            # c*exp(relu(s+t)) on ACT: Relu w/ t-bias ptr,
                            # then Exp w/ ln c bias
                            ar = mm_pool.tile(
                                [128, 2, ROWS], BF16, tag="ar", bufs=2
                            )
                            for v in range(2):
                                jt = 2 * t + v
                                nc.scalar.activation(
                                    out=ar[:, v, :], in_=s_bc, func=Act.Relu,
                                    bias=t_cols[:, jt : jt + 1],
                                )
                                nc.scalar.activation(
                                    out=z8[:, v, :], in_=ar[:, v, :],
                                    func=Act.Exp, bias=lncB,
                                )
                        else:
                            # z8 = fp8 max(es*et, c) at 2x
                            for v in range(2):
                                jt = 2 * t + v
                                nc.vector.tensor_scalar(
                                    out=z8[:, v, :], in0=es_bc,
                                    scalar1=et_cols[:, jt : jt + 1],
                                    scalar2=C, op0=Alu.mult, op1=Alu.max,
                                )
                        # mask applied as uint32 AND (mask bytes 00/FF)
                        nc.vector.tensor_tensor(
                            out=p8.bitcast(U32), in0=z8.bitcast(U32),
                            in1=mksl.bitcast(U32), op=Alu.bitwise_and,
                        )
                    wh_p = wh_all[:, 2 * t : 2 * t + 2, :]
                    for h in range(2):
                        sl = slice(512 * h, 512 * (h + 1))
                        nc.tensor.matmul(
                            psA[:, sl], wh_p, p8[:, :, sl],
                            start=first, stop=last,
                            perf_mode=DR, skip_group_check=True,
                        )
                        nc.tensor.matmul(
                            ps_den[:, sl], ones_m, p8[:, :, sl],
                            start=first, stop=last,
                            perf_mode=DR, skip_group_check=True,
                        )

                emit_dma(0)
                emit_dma(1)
                emit_wh(0)
                for g in range(NG):
                    if g + 2 < NG:
                        emit_dma(g + 2)
                    if g + 1 < NG:
                        emit_wh(g + 1)
                    for t in range(4 * g, 4 * g + 4):
                        emit_pair(t)

            # ---------------- epilogue ----------------
            num_sb = epi.tile([128, ROWS], FP32, tag="num")
            nc.scalar.copy(num_sb, psA)
            den_c = epi.tile([1, ROWS], FP32, tag="denc")
            nc.vector.tensor_copy(den_c, ps_den[0:1, :])

            with tc.tile_pool(name="psE", bufs=1, space="PSUM") as psE:
                ones1 = epi.tile([1, 1], FP32, tag="ones1")
                nc.vector.memset(ones1, 1.0)
                ps_dc = psE.tile([128, 8], FP32, tag="dc")
                for q in range(8):
                    nc.tensor.matmul(
                        ps_dc[:, q : q + 1],
                        den_c[:, 128 * q : 128 * (q + 1)], ones1,
                        start=True, stop=True, skip_group_check=True,
                    )
                rec_cols = epi.tile([128, 8], FP32, tag="rec")
                nc.vector.reciprocal(rec_cols, ps_dc)
                of_all = epi.tile([128, 8, F_OUT], FP32, tag="of")
                for qq in range(4):
                    ps_f = psE.tile([128, 2, 128], FP32, tag="f", bufs=2)
                    m0 = epi.tile([128, 2, 128], FP32, tag=f"m0_{qq % 2}")
                    r0 = epi.tile([128, 2, 128], FP32, tag=f"r0_{qq % 2}")
                    for v in range(2):
                        q = 2 * qq + v
                        nc.tensor.transpose(
                            ps_f[:, v, :], num_sb[:, 128 * q : 128 * (q + 1)],
                            idn,
                        )
                        nc.vector.tensor_scalar(
                            out=m0[:, v, :], in0=ps_f[:, v, :],
                            scalar1=rec_cols[:, q : q + 1],
                            scalar2=0.0, op0=Alu.mult, op1=Alu.min,
                        )
                        nc.scalar.activation(
                            out=r0[:, v, :], in_=ps_f[:, v, :],
                            func=Act.Relu, scale=rec_cols[:, q : q + 1],
                        )
                    e0 = epi.tile([128, 2, 128], FP32, tag=f"e0_{qq % 2}")
                    nc.scalar.activation(out=e0, in_=m0, func=Act.Exp)
                    nc.vector.scalar_tensor_tensor(
                        out=of_all[:, 2 * qq : 2 * qq + 2, :], in0=e0,
                        scalar=-1.0, in1=r0, op0=Alu.add, op1=Alu.add,
                    )
                    nc.sync.dma_start(
                        out=out_d[qq * 256 : (qq + 1) * 256, :].rearrange(
                            "(v p) f -> p v f", p=128
                        ),
                        in_=of_all[:, 2 * qq : 2 * qq + 2, :],
                    )


def kernel(X, A, weight, a, _trace=False, _tmpdir=None):
    X = np.ascontiguousarray(np.asarray(X, dtype=np.float32))
    A = np.ascontiguousarray(np.asarray(A, dtype=np.int32))
    weight = np.ascontiguousarray(np.asarray(weight, dtype=np.float32))
    a = np.ascontiguousarray(np.asarray(a, dtype=np.float32))

    if "nc" not in _cache:
        _cache["nc"] = _build()
    nc = _cache["nc"]

    bf16 = ml_dtypes.bfloat16
    fp8 = ml_dtypes.float8_e4m3

    Xbf = X.astype(bf16)
    # XTg[g, k, p, c] = X[1024 g + c, 128 k + p]
    XTg_base = np.ascontiguousarray(
        Xbf.reshape(NG, ROWS, KC, 128).transpose(0, 2, 3, 1)
    )
    w_bf = weight.astype(bf16)
    ident = np.eye(128, dtype=np.float32)

    # per-j-row mask byte in LOCAL (rotated) pair order: C-pair rows carry
    # fp8({0, 1}) for the Pool multiply, others {00, FF} for the AND
    cbyte = np.float32(1.0).astype(fp8).view(np.uint8).item()
    row_byte = np.empty(N, np.uint8)
    for t in range(NP):
        row_byte[256 * t : 256 * (t + 1)] = (
            cbyte if t in POOL_PAIRS else 0xFF
        )

    in_maps = []
    for c in range(N_CORES):
        i0 = c * ROWS
        # rotate groups so group 0 is this core's own rows; mask rows
        # follow the same j-permutation
        perm = [(c + g) % NG for g in range(NG)]
        XTg = np.ascontiguousarray(XTg_base[perm])
        edge = (A[i0 : i0 + ROWS].T > 0)                     # [N(j), ROWS(i)]
        edge_rot = np.ascontiguousarray(
            edge.reshape(NG, ROWS, ROWS)[perm].reshape(N, ROWS)
        )
        MK8 = np.ascontiguousarray(
            np.where(edge_rot, row_byte[:, None], 0).astype(np.uint8)
        ).view(fp8)
        in_maps.append(
            {
                "XTg": XTg,
                "MK8": MK8,
                "w_in": w_bf,
                "a_vec": a,
                "ident": ident,
            }
        )

    res = run_bass_kernel_spmd(
        nc, in_maps, core_ids=list(range(N_CORES)), trace=_trace, tmpdir=_tmpdir
    )
    out = np.concatenate([res.results[c]["out"] for c in range(N_CORES)], axis=0)
    if _trace:
        kernel._last_results = res
    return out


# revision 3
# speedup vs baseline: 1.0064x; 1.0064x over previous
"""GAT layer (nn_GATLayer) on 8 Trainium2 NeuronCores — AND-mask formulation.

Math (reference):
    Wh = X @ weight;  s = Wh @ a[:F];  t = Wh @ a[F:]
    att = softmax(where(A > 0, relu(s_i + t_j), -9e15), axis=1)
    out = elu(att @ Wh)

Kernel formulation (shift-free softmax, exact up to fp rounding):
    es_i = c*exp(s_i) (bf16 row), et_j = exp(t_j) (fp32 col)
    p_ij = A_ij * max(es_i*et_j, c)    [= c*e^{relu(s+t)} on edges]
    out_i = elu((p_i: @ Wh) / sum_j p_ij)

Per-pair mask application, three ISA-valid engine paths:
  R-pairs (DVE): z8 = fp8 max(es*et, c) via tensor_scalar 2x_2p, then
    p8 = z8 & mask as a uint32 bitwise AND (4 fp8 lanes per u32 lane,
    quarter free-size) — mask bytes are {0x00, 0xFF}.
  A-pairs (ACT+DVE): z8 = fp8 c*exp(relu(s_bc + t_j)) via two ACT ops
    (Relu with t-bias ptr, then Exp with ln c bias), then the same AND.
  C-pairs (DVE+Pool): rt = bf16 max(es*et, c) via tensor_scalar 4x, then
    p8 = rt * mask on Pool (tensor_tensor mult) — mask is {0, 1} so all
    pair classes produce identically-scaled p8.
The host encodes the mask slab per pair class (byte-mask rows for R/A
pairs, {0,c} fp8 rows for C-pairs) — dtype/layout prep only. Mask DMAs
are per-pair for fine-grained pipeline flow.

num = Wh8^T @ p8 and den = 1^T @ p8 accumulate in PSUM over all 64
j-tiles (fp8 DoubleRow, 4 matmuls per pair); epilogue is reciprocal +
PE transposes + fused ELU, output DMA'd per 256-row slab.

Sharding: 1D row partition across 8 cores (1024 rows each); X^T bf16
grouped + rotated so group 0 is own rows; weight bf16 replicated.
"""

import numpy as np
import ml_dtypes

import concourse.bass as bass
import concourse.bacc as bacc
import concourse.mybir as mybir
import concourse.tile as tile
from concourse.bass_utils import run_bass_kernel_spmd

N = 8192
F_IN = 512
F_OUT = 128
N_CORES = 8
ROWS = N // N_CORES          # 1024 rows per core
NT = N // 128                # 64 j tiles
NP = NT // 2                 # 32 j tile pairs
KC = F_IN // 128             # 4 f_in chunks
NG = 8                       # XT groups (8 j-tiles each)

C = 0.25                     # clamp value / global scale
LNC = float(np.log(C))

FP32 = mybir.dt.float32
BF16 = mybir.dt.bfloat16
FP8 = mybir.dt.float8e4
U32 = mybir.dt.uint32
Alu = mybir.AluOpType
Act = mybir.ActivationFunctionType
DR = mybir.MatmulPerfMode.DoubleRow

# engine assignment per pair (tuned against TimelineSim). Pool pairs are
# front-loaded: their masks arrive early and Pool's slow multiplies must
# not become the tail straggler.
POOL_PAIRS = frozenset({2, 5, 7, 11, 14, 17, 20, 23, 26, 29})
ACT_PAIRS = frozenset({3, 8, 13, 18, 24, 28})

_cache = {}


def _build():
    nc = bacc.Bacc("TRN2", target_bir_lowering=False, debug=False,
                   num_devices=N_CORES)

    XTg = nc.dram_tensor("XTg", [NG, KC, 128, ROWS], BF16, kind="ExternalInput")
    MK8 = nc.dram_tensor("MK8", [N, ROWS], FP8, kind="ExternalInput")
    w_in = nc.dram_tensor("w_in", [F_IN, F_OUT], BF16, kind="ExternalInput")
    a_vec = nc.dram_tensor("a_vec", [2 * F_OUT, 1], FP32, kind="ExternalInput")
    ident = nc.dram_tensor("ident", [128, 128], FP32, kind="ExternalInput")
    out_d = nc.dram_tensor("out", [ROWS, F_OUT], FP32, kind="ExternalOutput")

    with tile.TileContext(nc) as tc:
        _body(nc, tc, XTg, MK8, w_in, a_vec, ident, out_d)

    nc.compile()
    return nc


def _body(nc, tc, XTg, MK8, w_in, a_vec, ident, out_d):
    with (
        tc.tile_pool(name="setup", bufs=1) as setup,
        tc.tile_pool(name="xtg", bufs=4) as xtg_pool,
        tc.tile_pool(name="mk", bufs=1) as mk_pool,
        tc.tile_pool(name="mm", bufs=10) as mm_pool,
        tc.tile_pool(name="epi", bufs=1) as epi,
    ):
        # ---------------- setup: weights ----------------
        psW_cm = tc.tile_pool(name="psW", bufs=1, space="PSUM")
        psW = psW_cm.__enter__()

        w_sb = setup.tile([128, KC, 128], BF16)
        nc.sync.dma_start(
            out=w_sb, in_=w_in.rearrange("(k p) f -> p k f", p=128)
        )
        idn = setup.tile([128, 128], FP32)
        nc.sync.dma_start(out=idn, in_=ident[:, :])
        a_cat = setup.tile([128, 2], BF16)
        nc.gpsimd.dma_start(
            out=a_cat, in_=a_vec.rearrange("(h p) o -> p (h o)", p=128)
        )
        idn_bf = setup.tile([128, 128], BF16)
        nc.vector.tensor_copy(idn_bf, idn)

        # first XT group doubles as own-rows X^T (host rotates groups)
        xtg0 = xtg_pool.tile([128, KC, ROWS], BF16, tag="g0")
        for k in range(KC):
            nc.sync.dma_start(
                out=xtg0[:, k, :],
                in_=XTg[0].rearrange("k p i -> p k i")[:, k, :],
            )

        # w_all[:, k, :] = [weight_k | w_t_k | w_s_k]  [128, KC, 130]
        w_all = setup.tile([128, KC, 130], BF16)
        for k in range(KC):
            nc.vector.tensor_copy(w_all[:, k, 0:F_OUT], w_sb[:, k, :])
            ps_wT = psW.tile([128, 128], BF16, tag="wT")
            nc.tensor.transpose(ps_wT, w_sb[:, k, :], idn_bf)
            wT = setup.tile([128, 128], BF16, tag=f"wT{k}")
            nc.vector.tensor_copy(wT, ps_wT)
            ps_a = psW.tile([128, 2], FP32, tag="pa")
            nc.tensor.matmul(ps_a, wT, a_cat, start=True, stop=True)
            # col F_OUT = w_t (a[F:]), col F_OUT+1 = w_s (a[:F])
            nc.vector.tensor_copy(w_all[:, k, F_OUT : F_OUT + 1], ps_a[:, 1:2])
            nc.vector.tensor_copy(
                w_all[:, k, F_OUT + 1 : F_OUT + 2], ps_a[:, 0:1]
            )
        psW_cm.__exit__(None, None, None)

        # ---------------- s (own rows) + es/s broadcasts ----------------
        psA_cm = tc.tile_pool(name="psAset", bufs=1, space="PSUM")
        psAs = psA_cm.__enter__()

        ps_s = psAs.tile([1, ROWS], FP32, tag="ps_s")
        for h in range(2):
            sl = slice(512 * h, 512 * (h + 1))
            for k in range(KC):
                nc.tensor.matmul(
                    ps_s[:, sl],
                    w_all[:, k, 129:130],
                    xtg0[:, k, sl],
                    start=(k == 0), stop=(k == KC - 1),
                    skip_group_check=True,
                )
        lnc1 = setup.tile([1, 1], FP32, tag="lnc1")
        nc.vector.memset(lnc1, LNC)
        es_row = setup.tile([1, ROWS], BF16)
        s_row = setup.tile([1, ROWS], BF16)
        for h in range(2):
            sl = slice(512 * h, 512 * (h + 1))
            nc.scalar.activation(
                out=es_row[:, sl], in_=ps_s[:, sl], func=Act.Exp, bias=lnc1
            )
            nc.vector.tensor_copy(s_row[:, sl], ps_s[:, sl])
        ones_r = setup.tile([1, 128], BF16)
        nc.vector.memset(ones_r, 1.0)
        es_bc = setup.tile([128, ROWS], BF16)
        s_bc = setup.tile([128, ROWS], BF16)
        for h in range(2):
            sl = slice(512 * h, 512 * (h + 1))
            ps_b = psAs.tile([128, 512], FP32, tag="bc", bufs=2)
            nc.tensor.matmul(ps_b, ones_r, es_row[:, sl], start=True, stop=True)
            nc.vector.tensor_copy(es_bc[:, sl], ps_b)
            ps_b2 = psAs.tile([128, 512], FP32, tag="bc", bufs=2)
            nc.tensor.matmul(ps_b2, ones_r, s_row[:, sl], start=True, stop=True)
            nc.scalar.copy(s_bc[:, sl], ps_b2)
        psA_cm.__exit__(None, None, None)

        # den stationary: ones in 32 columns -> den sums land in psum
        # partitions 0-31 (only row 0 is read)
        ones_m = setup.tile([128, 2, 32], FP8)
        nc.vector.memset(ones_m, 1.0)

        wh_all = setup.tile([128, NT, F_OUT], FP8)
        et_cols = setup.tile([128, NT], FP32)
        t_cols = setup.tile([128, NT], FP32)
        lncB = setup.tile([128, 1], FP32, tag="lncB")
        nc.vector.memset(lncB, LNC)

        with (
            tc.tile_pool(name="psA", bufs=1, space="PSUM") as psAp,
            tc.tile_pool(name="psDen", bufs=1, space="PSUM") as psDp,
        ):
            psA = psAp.tile([128, ROWS], FP32)
            ps_den = psDp.tile([32, ROWS], FP32)

            with tc.tile_pool(name="psS", bufs=1, space="PSUM") as psS:
                mk_tiles = {}
                xtg_tiles = {0: xtg0}

                def emit_dma(g):
                    if g > 0:
                        xtg = xtg_pool.tile([128, KC, ROWS], BF16)
                        xtg_tiles[g] = xtg
                        for k in range(KC):
                            nc.sync.dma_start(
                                out=xtg[:, k, :],
                                in_=XTg[g].rearrange("k p i -> p k i")[:, k, :],
                            )
                    for t in range(4 * g, 4 * g + 4):
                        mk = mk_pool.tile([128, 2, ROWS], FP8, tag="mk", bufs=16)
                        mk_tiles[t] = mk
                        nc.sync.dma_start(
                            out=mk,
                            in_=MK8[256 * t : 256 * (t + 1), :].rearrange(
                                "(t p) i -> p t i", p=128
                            ),
                        )

                def emit_wh(g):
                    # Wh | t for the group's 8 tiles (ps_p must stay within
                    # one 2KB PSUM bank: [128, 2, 130] = 1040B)
                    xtg = xtg_tiles[g]
                    for q in range(4):
                        jt0 = 8 * g + 2 * q
                        ps_p = psS.tile([128, 2, 130], FP32, bufs=4)
                        for v in range(2):
                            co = 128 * (2 * q + v)
                            for k in range(KC):
                                nc.tensor.matmul(
                                    ps_p[:, v, :],
                                    xtg[:, k, co : co + 128],
                                    w_all[:, k, :],
                                    start=(k == 0), stop=(k == KC - 1),
                                    skip_group_check=True,
                                )
                        nc.scalar.copy(
                            wh_all[:, jt0 : jt0 + 2, :], ps_p[:, :, 0:F_OUT]
                        )
                        nc.scalar.activation(
                            out=et_cols[:, jt0 : jt0 + 2],
                            in_=ps_p[:, :, F_OUT : F_OUT + 1],
                            func=Act.Exp,
                        )
                        nc.scalar.activation(
                            out=t_cols[:, jt0 : jt0 + 2],
                            in_=ps_p[:, :, F_OUT : F_OUT + 1],
                            func=Act.Identity,
                        )

                def emit_pair(t):
                    mksl = mk_tiles[t]
                    first, last = t == 0, t == NP - 1
                    p8 = mm_pool.tile([128, 2, ROWS], FP8, tag="p8", bufs=14)
                    if t in POOL_PAIRS:
                        # rt = bf16 max(es*et, c) at 4x; Pool multiplies
                        # by the {0, c} mask
                        rt = mm_pool.tile([128, 2, ROWS], BF16, tag="rt", bufs=3)
                        for v in range(2):
                            jt = 2 * t + v
                            nc.vector.tensor_scalar(
                                out=rt[:, v, :], in0=es_bc,
                                scalar1=et_cols[:, jt : jt + 1],
                                scalar2=C, op0=Alu.mult, op1=Alu.max,
                            )
                        nc.gpsimd.tensor_tensor(
                            out=p8, in0=rt, in1=mksl, op=Alu.mult
                        )
                    else:
                        z8 = mm_pool.tile([128, 2, ROWS], FP8, tag="z8", bufs=5)
                        if t in ACT_PAIRS:
                            # c*exp(relu(s+t)) on ACT: Relu w/ t-bias ptr,
                            # then Exp w/ ln c bias
                            ar = mm_pool.tile(
                                [128, 2, ROWS], BF16, tag="ar", bufs=2
                            )
                            for v in range(2):
                                jt = 2 * t + v
                                nc.scalar.activation(
                                    out=ar[:, v, :], in_=s_bc, func=Act.Relu,
                                    bias=t_cols[:, jt : jt + 1],
                                )
                                nc.scalar.activation(
                                    out=z8[:, v, :], in_=ar[:, v, :],
                                    func=Act.Exp, bias=lncB,
                                )
                        else:
                            # z8 = fp8 max(es*et, c) at 2x
                            for v in range(2):
                                jt = 2 * t + v
                                nc.vector.tensor_scalar(
                                    out=z8[:, v, :], in0=es_bc,
                                    scalar1=et_cols[:, jt : jt + 1],
                                    scalar2=C, op0=Alu.mult, op1=Alu.max,
                                )
                        # mask applied as uint32 AND (mask bytes 00/FF)
                        nc.vector.tensor_tensor(
                            out=p8.bitcast(U32), in0=z8.bitcast(U32),
                            in1=mksl.bitcast(U32), op=Alu.bitwise_and,
                        )
                    wh_p = wh_all[:, 2 * t : 2 * t + 2, :]
                    for h in range(2):
                        sl = slice(512 * h, 512 * (h + 1))
                        nc.tensor.matmul(
                            psA[:, sl], wh_p, p8[:, :, sl],
                            start=first, stop=last,
                            perf_mode=DR, skip_group_check=True,
                        )
                        nc.tensor.matmul(
                            ps_den[:, sl], ones_m, p8[:, :, sl],
                            start=first, stop=last,
                            perf_mode=DR, skip_group_check=True,
                        )

                emit_dma(0)
                emit_dma(1)
                emit_wh(0)
                for g in range(NG):
                    if g + 2 < NG:
                        emit_dma(g + 2)
                    if g + 1 < NG:
                        emit_wh(g + 1)
                    for t in range(4 * g, 4 * g + 4):
                        emit_pair(t)

            # ---------------- epilogue ----------------
            num_sb = epi.tile([128, ROWS], FP32, tag="num")
            nc.scalar.copy(num_sb, psA)
            den_c = epi.tile([1, ROWS], FP32, tag="denc")
            nc.vector.tensor_copy(den_c, ps_den[0:1, :])

            with tc.tile_pool(name="psE", bufs=1, space="PSUM") as psE:
                ones1 = epi.tile([1, 1], FP32, tag="ones1")
                nc.vector.memset(ones1, 1.0)
                ps_dc = psE.tile([128, 8], FP32, tag="dc")
                for q in range(8):
                    nc.tensor.matmul(
                        ps_dc[:, q : q + 1],
                        den_c[:, 128 * q : 128 * (q + 1)], ones1,
                        start=True, stop=True, skip_group_check=True,
                    )
                rec_cols = epi.tile([128, 8], FP32, tag="rec")
                nc.vector.reciprocal(rec_cols, ps_dc)
                of_all = epi.tile([128, 8, F_OUT], FP32, tag="of")
                for qq in range(4):
                    ps_f = psE.tile([128, 2, 128], FP32, tag="f", bufs=2)
                    m0 = epi.tile([128, 2, 128], FP32, tag=f"m0_{qq % 2}")
                    r0 = epi.tile([128, 2, 128], FP32, tag=f"r0_{qq % 2}")
                    for v in range(2):
                        q = 2 * qq + v
                        nc.tensor.transpose(
                            ps_f[:, v, :], num_sb[:, 128 * q : 128 * (q + 1)],
                            idn,
                        )
                        nc.vector.tensor_scalar(
                            out=m0[:, v, :], in0=ps_f[:, v, :],
                            scalar1=rec_cols[:, q : q + 1],
                            scalar2=0.0, op0=Alu.mult, op1=Alu.min,
                        )
                        nc.scalar.activation(
                            out=r0[:, v, :], in_=ps_f[:, v, :],
                            func=Act.Relu, scale=rec_cols[:, q : q + 1],
                        )
                    e0 = epi.tile([128, 2, 128], FP32, tag=f"e0_{qq % 2}")
                    nc.scalar.activation(out=e0, in_=m0, func=Act.Exp)
                    nc.vector.scalar_tensor_tensor(
                        out=of_all[:, 2 * qq : 2 * qq + 2, :], in0=e0,
                        scalar=-1.0, in1=r0, op0=Alu.add, op1=Alu.add,
                    )
                    nc.sync.dma_start(
                        out=out_d[qq * 256 : (qq + 1) * 256, :].rearrange(
                            "(v p) f -> p v f", p=128
                        ),
                        in_=of_all[:, 2 * qq : 2 * qq + 2, :],
                    )


def kernel(X, A, weight, a, _trace=False, _tmpdir=None):
    X = np.ascontiguousarray(np.asarray(X, dtype=np.float32))
    A = np.ascontiguousarray(np.asarray(A, dtype=np.int32))
    weight = np.ascontiguousarray(np.asarray(weight, dtype=np.float32))
    a = np.ascontiguousarray(np.asarray(a, dtype=np.float32))

    if "nc" not in _cache:
        _cache["nc"] = _build()
    nc = _cache["nc"]

    bf16 = ml_dtypes.bfloat16
    fp8 = ml_dtypes.float8_e4m3

    Xbf = X.astype(bf16)
    # XTg[g, k, p, c] = X[1024 g + c, 128 k + p]
    XTg_base = np.ascontiguousarray(
        Xbf.reshape(NG, ROWS, KC, 128).transpose(0, 2, 3, 1)
    )
    w_bf = weight.astype(bf16)
    ident = np.eye(128, dtype=np.float32)

    # per-j-row mask byte in LOCAL (rotated) pair order: C-pair rows carry
    # fp8({0, 1}) for the Pool multiply, others {00, FF} for the AND
    cbyte = np.float32(1.0).astype(fp8).view(np.uint8).item()
    row_byte = np.empty(N, np.uint8)
    for t in range(NP):
        row_byte[256 * t : 256 * (t + 1)] = (
            cbyte if t in POOL_PAIRS else 0xFF
        )

    in_maps = []
    for c in range(N_CORES):
        i0 = c * ROWS
        # rotate groups so group 0 is this core's own rows; mask rows
        # follow the same j-permutation
        perm = [(c + g) % NG for g in range(NG)]
        XTg = np.ascontiguousarray(XTg_base[perm])
        edge = (A[i0 : i0 + ROWS].T > 0)                     # [N(j), ROWS(i)]
        edge_rot = np.ascontiguousarray(
            edge.reshape(NG, ROWS, ROWS)[perm].reshape(N, ROWS)
        )
        MK8 = np.ascontiguousarray(
            np.where(edge_rot, row_byte[:, None], 0).astype(np.uint8)
        ).view(fp8)
        in_maps.append(
            {
                "XTg": XTg,
                "MK8": MK8,
                "w_in": w_bf,
                "a_vec": a,
                "ident": ident,
            }
        )

    res = run_bass_kernel_spmd(
        nc, in_maps, core_ids=list(range(N_CORES)), trace=_trace, tmpdir=_tmpdir
    )
    out = np.concatenate([res.results[c]["out"] for c in range(N_CORES)], axis=0)
    if _trace:
        kernel._last_results = res
    return out
